# revision 32
# baseline (speedup 1.0000x reference)
"""CapsuleModel2 segment-reduce kernel for 8 TRN2 NeuronCores (v4).

Math (per reference.py):
    feats  = class_capsules.reshape(65536, 272)[point_idx]        # [P, 272]
    sums   = segment_sum(feats, segment_ids, 4096)                # [4096, 272]
    counts = segment_sum(ones)                                    # [4096]
    out    = sigmoid((sums / max(counts,1)) @ W + b)              # [4096, 19]

Key identity: (sums @ W) = segment_sum(feats @ W) — project the 65536x272
grid down to 19 channels + a constant count column FIRST (fp8 matmul on the
PE), write the 256B-row table to DRAM, then dma_gather one row per point.

Distribution (table-sharded): core k owns grid cells [k*8192,(k+1)*8192) and
the points hitting them; partial sums over ALL 4096 segments; one
ReduceScatter(add). Points are binned per 64-segment window on host (window
padded to whole 128-point chunks) so each chunk's one-hot matmul
psum[64, 20] += oh^T @ rows covers one window.

v4 vs the 530us v1 baseline (same gather mechanism — SWDGE dma_gather at
~17ns/packet/queue is the floor):
  - 16 slices round-robin on all 4 SWDGE queues for steady transfer flow
  - one-hot matrices precomputed on HOST and DMA'd in (bf16) — no DVE
    is_equal builds on the critical path
  - partial sums stored p-major [64seg%64, 64win, 20] f16 (contiguous 160B
    per partition per store, vs 40B-descriptor sprays), per-slice `part`
    tiles from a bufs=2 pool so slice i+1 never waits on slice i's store
  - ReduceScatter splits by partition rows: core k owns segments with
    s%64 in [8k,8k+8); host reorders on assemble
  - batched psum->part copy (one scalar op per slice, not per window)
"""

import sys

for _p in ('/opt/trn_rl_repo',):
    if _p not in sys.path:
        sys.path.insert(0, _p)

import numpy as np
import ml_dtypes

import concourse.bacc as bacc
import concourse.mybir as mybir
import concourse.tile as tile

BF16 = mybir.dt.bfloat16
F32 = mybir.dt.float32
I16 = mybir.dt.int16
F16 = mybir.dt.float16
F8 = mybir.dt.float8e4

NCORE = 8
GRID = 65536
GPC = GRID // NCORE          # 8192 grid cells per core
D = 272                      # capsule feature dim
NCH = 19                     # output channels
NW = NCH + 1                 # + count column
NSEG = 4096
WIN = 64                     # segments per window (one-hot width)
NWIN = NSEG // WIN           # 64 windows
CAP = 1152                   # padded points per (core, window); actual max ~1118
CPW = CAP // 128             # 9 chunks per window
NCHUNK = NWIN * CPW          # 576 chunks per core
NIDX = NCHUNK * 128          # 73728 gather slots per core
NSLICE = 32
CPS = NCHUNK // NSLICE       # 36 chunks per slice
WPS = NWIN // NSLICE         # 4 windows per slice
IDX_PER_SLICE = NIDX // NSLICE   # 4608
ELEM = 128                   # table row width (bf16) = 256B (dma_gather min)
MTILE = 2048                 # grid cells per projection step
WSCALE = 16.0                # fp8 W pre-scale; cancels against count col


def build_nc(skip_collective=False):
    nc = bacc.Bacc("TRN2", num_devices=NCORE, num_swdge_queues=4)

    gridT = nc.dram_tensor("gridT", [D, GPC], F8, kind="ExternalInput")
    w_pack = nc.dram_tensor("w_pack", [128, 60], F8, kind="ExternalInput")
    idx_in = nc.dram_tensor("idx", [128, NSLICE, IDX_PER_SLICE // 16], I16,
                            kind="ExternalInput")
    oh_in = nc.dram_tensor("oh", [128, NCHUNK, WIN], BF16,
                           kind="ExternalInput")
    bias_in = nc.dram_tensor("bias", [128, NCH], F32, kind="ExternalInput")
    if skip_collective:
        out_t = nc.dram_tensor("out", [64, NWIN, NW], F16,
                               kind="ExternalOutput")
    else:
        out_t = nc.dram_tensor("out", [8, NWIN, NCH], F32,
                               kind="ExternalOutput")

    table = nc.dram_tensor("table", [GPC, ELEM], BF16)
    partial_d = nc.dram_tensor("partial", [2, 64, NWIN // 2, NW], F16)
    rs_out = nc.dram_tensor("rs_out", [2, 8, NWIN // 2, NW], F16)

    with tile.TileContext(nc) as tc:
        with (
            tc.tile_pool(name="const", bufs=1) as cpool,
            tc.tile_pool(name="ohs", bufs=6) as opool,
            tc.tile_pool(name="grid", bufs=2) as gpool,
            tc.tile_pool(name="tab", bufs=2) as tpool,
            tc.tile_pool(name="ptab", bufs=2, space="PSUM") as pt_pool,
            tc.tile_pool(name="dst", bufs=6) as dpool,
            tc.tile_pool(name="pblk", bufs=4, space="PSUM") as pb_pool,
            tc.tile_pool(name="part", bufs=4) as papool,
            tc.tile_pool(name="fin", bufs=2) as fpool,
        ):
            # constants
            w_sb = cpool.tile([128, 60], F8)
            nc.sync.dma_start(w_sb[:], w_pack[:])
            bias_sb = cpool.tile([128, NCH], F32)
            nc.sync.dma_start(bias_sb[:], bias_in[:])
            idx_all = cpool.tile([128, NSLICE, IDX_PER_SLICE // 16], I16)
            nc.sync.dma_start(idx_all[:], idx_in[:])

            # ---- Phase A: projection -> DRAM table (256B bf16 rows) ----
            MS = MTILE // 128    # 16 psum chunks per mtile
            ksizes = [(0, 128), (128, 128), (256, 16)]
            for mt in range(GPC // MTILE):
                gt = gpool.tile([128, 3, MTILE], F8, tag="gt")
                for t, (k0, kn) in enumerate(ksizes):
                    nc.sync.dma_start(
                        gt[:kn, t, :],
                        gridT[k0:k0 + kn, mt * MTILE:(mt + 1) * MTILE])
                tab = tpool.tile([128, MS, ELEM], BF16, tag="tab")
                nc.vector.memset(tab[:, :, NCH:], 0.0)
                nc.vector.memset(tab[:, :, NCH:NW], WSCALE)
                for mq in range(MS // 4):
                    psum = pt_pool.tile([128, 4, NW], F32, tag="ptab")
                    for q in range(4):
                        ms = mq * 4 + q
                        for t, (k0, kn) in enumerate(ksizes):
                            nc.tensor.matmul(
                                out=psum[:, q, :],
                                lhsT=gt[:kn, t, ms * 128:(ms + 1) * 128],
                                rhs=w_sb[:kn, t * NW:(t + 1) * NW],
                                start=(t == 0), stop=(t == 2))
                    nc.scalar.copy(tab[:, mq * 4:(mq + 1) * 4, :NCH],
                                   psum[:, :, :NCH])
                # p-major table: grid cell c lives at table row
                # (c%128)*64 + c//128, so each partition stores MS
                # consecutive 256B rows in one descriptor
                nc.scalar.dma_start(
                    table[:].rearrange("(p r) e -> p r e", p=128)
                        [:, mt * MS:(mt + 1) * MS, :],
                    tab[:])

            # ---- Phase B: gather + one-hot reduce ----
            for s in range(NSLICE):
                # stream this slice's one-hot just-in-time (keeps the 9.4MB
                # oh transfer off the projection-phase DMA critical path)
                ohs = opool.tile([128, CPS, WIN], BF16, tag="ohs")
                nc.sync.dma_start(ohs[:], oh_in[:, s * CPS:(s + 1) * CPS, :])
                dst = dpool.tile([128, CPS, ELEM], BF16, tag="dst")
                nc.gpsimd.dma_gather(
                    dst[:], table[:], idx_all[:, s, :],
                    IDX_PER_SLICE, IDX_PER_SLICE, ELEM, single_packet=False,
                    queue_num=s % 4)
                psum_w = pb_pool.tile([WIN, WPS, NW], F32, tag="pblk")
                for c in range(CPS):
                    h, j = divmod(c, CPW)
                    nc.tensor.matmul(
                        out=psum_w[:, h, :],
                        lhsT=ohs[:, c, :],
                        rhs=dst[:, c, :NW],
                        start=(j == 0), stop=(j == CPW - 1))
                part = papool.tile([WIN, WPS, NW], F16, tag="part")
                nc.scalar.copy(part[:], psum_w[:])
                if skip_collective:
                    nc.sync.dma_start(out_t[:, WPS * s:WPS * (s + 1), :],
                                      part[:])
                else:
                    hh, sl = divmod(s, NSLICE // 2)
                    nc.sync.dma_start(
                        partial_d[hh, :, WPS * sl:WPS * (sl + 1), :],
                        part[:])
                # first-half RS trigger placed at s=24: by the time the Pool
                # engine reaches it, slices 0..15's stores are long done, so
                # its wait is pre-satisfied and cannot stall gather issue;
                # the RS runs concurrently with the remaining gather slices
                if not skip_collective and s == 3 * NSLICE // 4:
                    nc.gpsimd.collective_compute(
                        "ReduceScatter",
                        mybir.AluOpType.add,
                        replica_groups=[list(range(NCORE))],
                        ins=[partial_d[0]],
                        outs=[rs_out[0]],
                    )

            if not skip_collective:
                nc.gpsimd.collective_compute(
                    "ReduceScatter",
                    mybir.AluOpType.add,
                    replica_groups=[list(range(NCORE))],
                    ins=[partial_d[1]],
                    outs=[rs_out[1]],
                )

            # ---- Phase C: finalize (per RS half) ----
            if not skip_collective:
                HW2 = NWIN // 2
                for h in range(2):
                    wsl = slice(h * HW2, (h + 1) * HW2)
                    fin16 = fpool.tile([8, HW2, NW], F16, tag="fin16")
                    nc.sync.dma_start(fin16[:], rs_out[h])
                    fin = fpool.tile([8, HW2, NW], F32, tag="fin")
                    nc.vector.tensor_copy(fin[:], fin16[:])
                    cnt = fpool.tile([8, HW2, 1], F32, tag="cnt")
                    nc.vector.tensor_scalar_max(cnt[:], fin[:, :, NCH:NW], 1.0)
                    rec = fpool.tile([8, HW2, 1], F32, tag="rec")
                    nc.vector.reciprocal(rec[:], cnt[:])
                    sc = fpool.tile([8, HW2, NCH], F32, tag="sc")
                    nc.vector.tensor_tensor(
                        out=sc[:], in0=fin[:, :, :NCH],
                        in1=rec[:].broadcast_to([8, HW2, NCH]),
                        op=mybir.AluOpType.mult)
                    sc2 = fpool.tile([8, HW2, NCH], F32, tag="sc2")
                    nc.vector.tensor_tensor(
                        out=sc2[:], in0=sc[:],
                        in1=bias_sb[:8].rearrange("p (h c) -> p h c", h=1)
                            .broadcast_to([8, HW2, NCH]),
                        op=mybir.AluOpType.add)
                    og = fpool.tile([8, HW2, NCH], F32, tag="og")
                    nc.scalar.activation(og[:], sc2[:],
                                         mybir.ActivationFunctionType.Sigmoid)
                    nc.sync.dma_start(out_t[:, wsl, :], og[:])

    nc.compile()
    return nc


def prep_inputs(class_capsules, W, b, point_idx, segment_ids, num_segments=NSEG):
    """Host-side sharding: returns in_maps (list of 8 dicts)."""
    assert int(num_segments) == NSEG
    grid = np.ascontiguousarray(class_capsules.reshape(GRID, D), np.float32)
    point_idx = np.asarray(point_idx, np.int64)
    segment_ids = np.asarray(segment_ids, np.int64)
    W = np.asarray(W, np.float32)
    b = np.asarray(b, np.float32)

    f8 = ml_dtypes.float8_e4m3fn
    w_pack = np.zeros((128, 60), f8)
    w20 = np.concatenate([W, np.zeros((D, 1), np.float32)], 1) * WSCALE
    w_pack[:, 0:20] = w20[0:128].astype(f8)
    w_pack[:, 20:40] = w20[128:256].astype(f8)
    w_pack[0:16, 40:60] = w20[256:272].astype(f8)

    bias_rep = np.tile(b[None, :], (128, 1)).astype(np.float32)

    in_maps = []
    for k in range(NCORE):
        sel = (point_idx >= k * GPC) & (point_idx < (k + 1) * GPC)
        lidx = (point_idx[sel] - k * GPC).astype(np.int64)
        # p-major table layout: cell c lives at row (c%128)*64 + c//128
        lidx = ((lidx % 128) * (GPC // 128) + lidx // 128).astype(np.int16)
        lseg = segment_ids[sel]          # sorted ascending
        win = (lseg >> 6).astype(np.int64)
        srel = (lseg & 63).astype(np.float32)
        counts = np.bincount(win, minlength=NWIN)
        assert counts.max() <= CAP, f"core {k}: window count {counts.max()} > CAP"
        start = np.zeros(NWIN, np.int64)
        start[1:] = np.cumsum(counts)[:-1]
        rank = np.arange(lidx.size) - start[win]
        pos = win * CAP + rank

        idx_pad = np.zeros(NIDX, np.int16)
        srel_pad = np.full(NIDX, -1.0, np.float32)
        idx_pad[pos] = lidx
        srel_pad[pos] = srel

        # wrapped idx, contiguous per partition: [128, NSLICE, 288]
        idxw = np.empty((128, NSLICE, IDX_PER_SLICE // 16), np.int16)
        for s in range(NSLICE):
            chunk = idx_pad[s * IDX_PER_SLICE:(s + 1) * IDX_PER_SLICE]
            idxw[:, s, :] = np.tile(chunk.reshape(-1, 16).T, (8, 1))

        # host-built one-hot: oh[p, g, s] = (srel of slot g*128+p) == s
        sr = srel_pad.reshape(NCHUNK, 128)            # [chunk, slot]
        ohm = (sr.T[:, :, None] ==
               np.arange(WIN, dtype=np.float32)[None, None, :])
        oh = ohm.astype(ml_dtypes.bfloat16)           # [128, NCHUNK, 64]

        gridT_k = np.ascontiguousarray(
            grid[k * GPC:(k + 1) * GPC].T).astype(f8)

        in_maps.append({
            "gridT": gridT_k,
            "w_pack": w_pack,
            "idx": idxw,
            "oh": oh,
            "bias": bias_rep,
        })
    return in_maps


def assemble(results):
    # core k's rs slice holds segments s with s%64 in [8k, 8k+8):
    # out_k[r, w, :] = segment w*64 + 8k + r
    out = np.empty((NSEG, NCH), np.float32)
    for k in range(NCORE):
        res = results[k]["out"]           # [8, 64, 19]
        segs = (np.arange(NWIN)[None, :] * WIN + 8 * k
                + np.arange(8)[:, None])  # [8, 64]
        out[segs.ravel()] = res.reshape(-1, NCH)
    return out


_NC_CACHE = {}


def kernel(class_capsules, W, b, point_idx, segment_ids, num_segments):
    """Full-input entry point: shard across 8 NeuronCores, run, reassemble."""
    from concourse.bass_utils import run_bass_kernel_spmd

    in_maps = prep_inputs(np.asarray(class_capsules), np.asarray(W),
                          np.asarray(b), np.asarray(point_idx),
                          np.asarray(segment_ids), int(num_segments))
    if "nc" not in _NC_CACHE:
        _NC_CACHE["nc"] = build_nc()
    res = run_bass_kernel_spmd(_NC_CACHE["nc"], in_maps, list(range(NCORE)))
    return assemble(res.results)


# revision 34
# speedup vs baseline: 1.0016x; 1.0016x over previous
"""CapsuleModel2 segment-reduce kernel for 8 TRN2 NeuronCores (v4).

Math (per reference.py):
    feats  = class_capsules.reshape(65536, 272)[point_idx]        # [P, 272]
    sums   = segment_sum(feats, segment_ids, 4096)                # [4096, 272]
    counts = segment_sum(ones)                                    # [4096]
    out    = sigmoid((sums / max(counts,1)) @ W + b)              # [4096, 19]

Key identity: (sums @ W) = segment_sum(feats @ W) — project the 65536x272
grid down to 19 channels + a constant count column FIRST (fp8 matmul on the
PE), write the 256B-row table to DRAM, then dma_gather one row per point.

Distribution (table-sharded): core k owns grid cells [k*8192,(k+1)*8192) and
the points hitting them; partial sums over ALL 4096 segments; one
ReduceScatter(add). Points are binned per 64-segment window on host (window
padded to whole 128-point chunks) so each chunk's one-hot matmul
psum[64, 20] += oh^T @ rows covers one window.

v4 vs the 530us v1 baseline (same gather mechanism — SWDGE dma_gather at
~17ns/packet/queue is the floor):
  - 16 slices round-robin on all 4 SWDGE queues for steady transfer flow
  - one-hot matrices precomputed on HOST and DMA'd in (bf16) — no DVE
    is_equal builds on the critical path
  - partial sums stored p-major [64seg%64, 64win, 20] f16 (contiguous 160B
    per partition per store, vs 40B-descriptor sprays), per-slice `part`
    tiles from a bufs=2 pool so slice i+1 never waits on slice i's store
  - ReduceScatter splits by partition rows: core k owns segments with
    s%64 in [8k,8k+8); host reorders on assemble
  - batched psum->part copy (one scalar op per slice, not per window)
"""

import sys

for _p in ('/opt/trn_rl_repo',):
    if _p not in sys.path:
        sys.path.insert(0, _p)

import numpy as np
import ml_dtypes

import concourse.bacc as bacc
import concourse.mybir as mybir
import concourse.tile as tile

BF16 = mybir.dt.bfloat16
F32 = mybir.dt.float32
I16 = mybir.dt.int16
F16 = mybir.dt.float16
F8 = mybir.dt.float8e4

NCORE = 8
GRID = 65536
GPC = GRID // NCORE          # 8192 grid cells per core
D = 272                      # capsule feature dim
NCH = 19                     # output channels
NW = NCH + 1                 # + count column
NSEG = 4096
WIN = 64                     # segments per window (one-hot width)
NWIN = NSEG // WIN           # 64 windows
CAP = 1152                   # padded points per (core, window); actual max ~1118
CPW = CAP // 128             # 9 chunks per window
NCHUNK = NWIN * CPW          # 576 chunks per core
NIDX = NCHUNK * 128          # 73728 gather slots per core
NSLICE = 32
CPS = NCHUNK // NSLICE       # 36 chunks per slice
WPS = NWIN // NSLICE         # 4 windows per slice
IDX_PER_SLICE = NIDX // NSLICE   # 4608
ELEM = 128                   # table row width (bf16) = 256B (dma_gather min)
MTILE = 4096                 # grid cells per projection step
WSCALE = 16.0                # fp8 W pre-scale; cancels against count col


def build_nc(skip_collective=False):
    nc = bacc.Bacc("TRN2", num_devices=NCORE, num_swdge_queues=4)

    gridT = nc.dram_tensor("gridT", [D, GPC], F8, kind="ExternalInput")
    w_pack = nc.dram_tensor("w_pack", [128, 60], F8, kind="ExternalInput")
    idx_in = nc.dram_tensor("idx", [128, NSLICE, IDX_PER_SLICE // 16], I16,
                            kind="ExternalInput")
    oh_in = nc.dram_tensor("oh", [128, NCHUNK, WIN], BF16,
                           kind="ExternalInput")
    bias_in = nc.dram_tensor("bias", [128, NCH], F32, kind="ExternalInput")
    if skip_collective:
        out_t = nc.dram_tensor("out", [64, NWIN, NW], F16,
                               kind="ExternalOutput")
    else:
        out_t = nc.dram_tensor("out", [8, NWIN, NCH], F32,
                               kind="ExternalOutput")

    table = nc.dram_tensor("table", [GPC, ELEM], BF16)
    partial_d = nc.dram_tensor("partial", [2, 64, NWIN // 2, NW], F16)
    rs_out = nc.dram_tensor("rs_out", [2, 8, NWIN // 2, NW], F16)

    with tile.TileContext(nc) as tc:
        with (
            tc.tile_pool(name="const", bufs=1) as cpool,
            tc.tile_pool(name="ohs", bufs=4) as opool,
            tc.tile_pool(name="grid", bufs=2) as gpool,
            tc.tile_pool(name="tab", bufs=2) as tpool,
            tc.tile_pool(name="ptab", bufs=2, space="PSUM") as pt_pool,
            tc.tile_pool(name="dst", bufs=5) as dpool,
            tc.tile_pool(name="pblk", bufs=2, space="PSUM") as pb_pool,
            tc.tile_pool(name="part", bufs=2) as papool,
            tc.tile_pool(name="fin", bufs=2) as fpool,
        ):
            # constants
            w_sb = cpool.tile([128, 60], F8)
            nc.sync.dma_start(w_sb[:], w_pack[:])
            bias_sb = cpool.tile([128, NCH], F32)
            nc.sync.dma_start(bias_sb[:], bias_in[:])
            idx_all = cpool.tile([128, NSLICE, IDX_PER_SLICE // 16], I16)
            nc.sync.dma_start(idx_all[:], idx_in[:])

            # ---- Phase A: projection -> DRAM table (256B bf16 rows) ----
            MS = MTILE // 128    # 16 psum chunks per mtile
            ksizes = [(0, 128), (128, 128), (256, 16)]
            for mt in range(GPC // MTILE):
                gt = gpool.tile([128, 3, MTILE], F8, tag="gt")
                for t, (k0, kn) in enumerate(ksizes):
                    nc.sync.dma_start(
                        gt[:kn, t, :],
                        gridT[k0:k0 + kn, mt * MTILE:(mt + 1) * MTILE])
                tab = tpool.tile([128, MS, ELEM], BF16, tag="tab")
                nc.vector.memset(tab[:, :, NCH:], 0.0)
                nc.vector.memset(tab[:, :, NCH:NW], WSCALE)
                for mq in range(MS // 4):
                    psum = pt_pool.tile([128, 4, NW], F32, tag="ptab")
                    for q in range(4):
                        ms = mq * 4 + q
                        for t, (k0, kn) in enumerate(ksizes):
                            nc.tensor.matmul(
                                out=psum[:, q, :],
                                lhsT=gt[:kn, t, ms * 128:(ms + 1) * 128],
                                rhs=w_sb[:kn, t * NW:(t + 1) * NW],
                                start=(t == 0), stop=(t == 2))
                    nc.scalar.copy(tab[:, mq * 4:(mq + 1) * 4, :NCH],
                                   psum[:, :, :NCH])
                # p-major table: grid cell c lives at table row
                # (c%128)*64 + c//128, so each partition stores MS
                # consecutive 256B rows in one descriptor
                nc.scalar.dma_start(
                    table[:].rearrange("(p r) e -> p r e", p=128)
                        [:, mt * MS:(mt + 1) * MS, :],
                    tab[:])

            # ---- Phase B: gather + one-hot reduce ----
            for s in range(NSLICE):
                # stream this slice's one-hot just-in-time (keeps the 9.4MB
                # oh transfer off the projection-phase DMA critical path)
                ohs = opool.tile([128, CPS, WIN], BF16, tag="ohs")
                nc.sync.dma_start(ohs[:], oh_in[:, s * CPS:(s + 1) * CPS, :])
                dst = dpool.tile([128, CPS, ELEM], BF16, tag="dst")
                nc.gpsimd.dma_gather(
                    dst[:], table[:], idx_all[:, s, :],
                    IDX_PER_SLICE, IDX_PER_SLICE, ELEM, single_packet=False,
                    queue_num=s % 4)
                psum_w = pb_pool.tile([WIN, WPS, NW], F32, tag="pblk")
                for c in range(CPS):
                    h, j = divmod(c, CPW)
                    nc.tensor.matmul(
                        out=psum_w[:, h, :],
                        lhsT=ohs[:, c, :],
                        rhs=dst[:, c, :NW],
                        start=(j == 0), stop=(j == CPW - 1))
                part = papool.tile([WIN, WPS, NW], F16, tag="part")
                nc.scalar.copy(part[:], psum_w[:])
                if skip_collective:
                    nc.sync.dma_start(out_t[:, WPS * s:WPS * (s + 1), :],
                                      part[:])
                else:
                    hh, sl = divmod(s, NSLICE // 2)
                    nc.sync.dma_start(
                        partial_d[hh, :, WPS * sl:WPS * (sl + 1), :],
                        part[:])
                # first-half RS trigger placed at s=24: by the time the Pool
                # engine reaches it, slices 0..15's stores are long done, so
                # its wait is pre-satisfied and cannot stall gather issue;
                # the RS runs concurrently with the remaining gather slices
                if not skip_collective and s == 3 * NSLICE // 4:
                    nc.gpsimd.collective_compute(
                        "ReduceScatter",
                        mybir.AluOpType.add,
                        replica_groups=[list(range(NCORE))],
                        ins=[partial_d[0]],
                        outs=[rs_out[0]],
                    )

            if not skip_collective:
                nc.gpsimd.collective_compute(
                    "ReduceScatter",
                    mybir.AluOpType.add,
                    replica_groups=[list(range(NCORE))],
                    ins=[partial_d[1]],
                    outs=[rs_out[1]],
                )

            # ---- Phase C: finalize (per RS half) ----
            if not skip_collective:
                HW2 = NWIN // 2
                for h in range(2):
                    wsl = slice(h * HW2, (h + 1) * HW2)
                    fin16 = fpool.tile([8, HW2, NW], F16, tag="fin16")
                    nc.sync.dma_start(fin16[:], rs_out[h])
                    fin = fpool.tile([8, HW2, NW], F32, tag="fin")
                    nc.vector.tensor_copy(fin[:], fin16[:])
                    cnt = fpool.tile([8, HW2, 1], F32, tag="cnt")
                    nc.vector.tensor_scalar_max(cnt[:], fin[:, :, NCH:NW], 1.0)
                    rec = fpool.tile([8, HW2, 1], F32, tag="rec")
                    nc.vector.reciprocal(rec[:], cnt[:])
                    sc = fpool.tile([8, HW2, NCH], F32, tag="sc")
                    nc.vector.tensor_tensor(
                        out=sc[:], in0=fin[:, :, :NCH],
                        in1=rec[:].broadcast_to([8, HW2, NCH]),
                        op=mybir.AluOpType.mult)
                    sc2 = fpool.tile([8, HW2, NCH], F32, tag="sc2")
                    nc.vector.tensor_tensor(
                        out=sc2[:], in0=sc[:],
                        in1=bias_sb[:8].rearrange("p (h c) -> p h c", h=1)
                            .broadcast_to([8, HW2, NCH]),
                        op=mybir.AluOpType.add)
                    og = fpool.tile([8, HW2, NCH], F32, tag="og")
                    nc.scalar.activation(og[:], sc2[:],
                                         mybir.ActivationFunctionType.Sigmoid)
                    nc.sync.dma_start(out_t[:, wsl, :], og[:])

    nc.compile()
    return nc


def prep_inputs(class_capsules, W, b, point_idx, segment_ids, num_segments=NSEG):
    """Host-side sharding: returns in_maps (list of 8 dicts)."""
    assert int(num_segments) == NSEG
    grid = np.ascontiguousarray(class_capsules.reshape(GRID, D), np.float32)
    point_idx = np.asarray(point_idx, np.int64)
    segment_ids = np.asarray(segment_ids, np.int64)
    W = np.asarray(W, np.float32)
    b = np.asarray(b, np.float32)

    f8 = ml_dtypes.float8_e4m3fn
    w_pack = np.zeros((128, 60), f8)
    w20 = np.concatenate([W, np.zeros((D, 1), np.float32)], 1) * WSCALE
    w_pack[:, 0:20] = w20[0:128].astype(f8)
    w_pack[:, 20:40] = w20[128:256].astype(f8)
    w_pack[0:16, 40:60] = w20[256:272].astype(f8)

    bias_rep = np.tile(b[None, :], (128, 1)).astype(np.float32)

    in_maps = []
    for k in range(NCORE):
        sel = (point_idx >= k * GPC) & (point_idx < (k + 1) * GPC)
        lidx = (point_idx[sel] - k * GPC).astype(np.int64)
        # p-major table layout: cell c lives at row (c%128)*64 + c//128
        lidx = ((lidx % 128) * (GPC // 128) + lidx // 128).astype(np.int16)
        lseg = segment_ids[sel]          # sorted ascending
        win = (lseg >> 6).astype(np.int64)
        srel = (lseg & 63).astype(np.float32)
        counts = np.bincount(win, minlength=NWIN)
        assert counts.max() <= CAP, f"core {k}: window count {counts.max()} > CAP"
        start = np.zeros(NWIN, np.int64)
        start[1:] = np.cumsum(counts)[:-1]
        rank = np.arange(lidx.size) - start[win]
        pos = win * CAP + rank

        idx_pad = np.zeros(NIDX, np.int16)
        srel_pad = np.full(NIDX, -1.0, np.float32)
        idx_pad[pos] = lidx
        srel_pad[pos] = srel

        # wrapped idx, contiguous per partition: [128, NSLICE, 288]
        idxw = np.empty((128, NSLICE, IDX_PER_SLICE // 16), np.int16)
        for s in range(NSLICE):
            chunk = idx_pad[s * IDX_PER_SLICE:(s + 1) * IDX_PER_SLICE]
            idxw[:, s, :] = np.tile(chunk.reshape(-1, 16).T, (8, 1))

        # host-built one-hot: oh[p, g, s] = (srel of slot g*128+p) == s
        sr = srel_pad.reshape(NCHUNK, 128)            # [chunk, slot]
        ohm = (sr.T[:, :, None] ==
               np.arange(WIN, dtype=np.float32)[None, None, :])
        oh = ohm.astype(ml_dtypes.bfloat16)           # [128, NCHUNK, 64]

        gridT_k = np.ascontiguousarray(
            grid[k * GPC:(k + 1) * GPC].T).astype(f8)

        in_maps.append({
            "gridT": gridT_k,
            "w_pack": w_pack,
            "idx": idxw,
            "oh": oh,
            "bias": bias_rep,
        })
    return in_maps


def assemble(results):
    # core k's rs slice holds segments s with s%64 in [8k, 8k+8):
    # out_k[r, w, :] = segment w*64 + 8k + r
    out = np.empty((NSEG, NCH), np.float32)
    for k in range(NCORE):
        res = results[k]["out"]           # [8, 64, 19]
        segs = (np.arange(NWIN)[None, :] * WIN + 8 * k
                + np.arange(8)[:, None])  # [8, 64]
        out[segs.ravel()] = res.reshape(-1, NCH)
    return out


_NC_CACHE = {}


def kernel(class_capsules, W, b, point_idx, segment_ids, num_segments):
    """Full-input entry point: shard across 8 NeuronCores, run, reassemble."""
    from concourse.bass_utils import run_bass_kernel_spmd

    in_maps = prep_inputs(np.asarray(class_capsules), np.asarray(W),
                          np.asarray(b), np.asarray(point_idx),
                          np.asarray(segment_ids), int(num_segments))
    if "nc" not in _NC_CACHE:
        _NC_CACHE["nc"] = build_nc()
    res = run_bass_kernel_spmd(_NC_CACHE["nc"], in_maps, list(range(NCORE)))
    return assemble(res.results)
